# revision 9
# baseline (speedup 1.0000x reference)
"""NF4-style 4-bit quantized linear: out = x @ dequant(w).T on 8 TRN2 NeuronCores.

Column-parallel sharding: core c owns output features [c*512, (c+1)*512) and the
corresponding contiguous slices of the packed weight + quant state arrays; x is
replicated. The Tile scheduler serializes ALL DMA traffic against in-flight
xbar transposes, so every transfer lives on one serialized conveyor, fully
pinned here:
  [qw0, scales, qw1-4]  ->  x0  ->  wd0 wt0 wd1 wt1  ->  qw5  ->  x1  ->
  qw6 qw7  ->  wd2 wt2 ... wd7 wt7  ->  xt0 xt1 ... (steady 4MB x blocks)
Dequant (8 chunks of 4 k-tiles, 6 batched DVE ops each, triple-buffered) runs
in the shadow of the x transposes; each chunk round-trips through DRAM with an
xbar transpose into wT [k-partition, outf] layout. The first two 512-token
blocks form the ramp (8 psums, matmuls emitted in conveyor-readiness order);
warmup-matmul trains on zeroed tiles cover the pre-x0 window and the
x1-transpose gap so the PE's HAM clock gate never drops to half rate. Output
stores are batched per block; the last block stores per 128-token tile to
shorten the drain.
"""
import numpy as np

import concourse.bass as bass
import concourse.mybir as mybir
import concourse.tile as tile
from concourse import bacc
from concourse.tile_rust import add_dep_helper as tile_rust_add_dep
from concourse.bass_utils import run_bass_kernel_spmd

F16 = mybir.dt.float16
F32 = mybir.dt.float32
I32 = mybir.dt.int32
I16 = mybir.dt.int16
Alu = mybir.AluOpType

P = 128
TOKENS = 8192
IN_F = 4096
OUT_F = 4096
N_CORES = 8
O_C = OUT_F // N_CORES          # 512 out features per core
KT = IN_F // P                  # 32 k-tiles
BPR = IN_F // 2                 # 2048 packed bytes per weight row
NB_O = O_C // P                 # 4 o-tiles of 128 rows
TB = 512                        # token block (4MB transpose)

NKC = 8                         # W chunks
KKC = KT // NKC                 # 4 k-tiles per chunk
KCW = IN_F // NKC               # 512 k values per chunk
BCC = BPR // NKC                # 256 packed bytes per chunk (per row)
NBC = BCC // 32                 # 8 quant blocks per chunk (per row)

NRB = 2                         # ramp blocks (2*512 tokens = 8 psums)

N_WARM0 = 140                   # warmup matmuls before the first real matmul
N_WARM1 = 60                    # warmup matmuls covering the x1 transpose gap

# conveyor positions (for readiness-ordering the ramp matmuls):
# x0=0, wt0=2, wt1=4, x1=6, wt2=10, wt3=12, wt4=14, ... wt7=20
X_POS = [0, 6]
WT_POS = [2, 4, 10, 12, 14, 16, 18, 20]


def _build(tokens=TOKENS):
    nc = bacc.Bacc("TRN2", target_bir_lowering=False, debug=False,
                   enable_asserts=False)

    x = nc.dram_tensor("x", [tokens, IN_F], F16, kind="ExternalInput").ap()
    qw = nc.dram_tensor("qw", [O_C, BPR], I32, kind="ExternalInput").ap()
    qam = nc.dram_tensor("qam", [O_C, 64], I32, kind="ExternalInput").ap()
    qcode = nc.dram_tensor("qcode", [O_C, 64], F32, kind="ExternalInput").ap()
    qoff = nc.dram_tensor("qoff", [O_C, 64], F32, kind="ExternalInput").ap()
    am2 = nc.dram_tensor("am2", [O_C, 16], F32, kind="ExternalInput").ap()
    c2 = nc.dram_tensor("c2", [O_C, 16], F32, kind="ExternalInput").ap()
    out = nc.dram_tensor("out", [tokens, O_C], F16, kind="ExternalOutput").ap()

    n_steady = tokens // TB - NRB

    with tile.TileContext(nc) as tc:
        with tc.tile_pool(name="wt_pool", bufs=1) as wt_pool, \
             tc.tile_pool(name="wdram", bufs=1, space="DRAM") as wdram, \
             tc.tile_pool(name="sc_pool", bufs=1) as sc_pool, \
             tc.tile_pool(name="dq", bufs=2) as dq, \
             tc.tile_pool(name="xt_pool", bufs=2) as xt_pool, \
             tc.tile_pool(name="ps_pool", bufs=8, space="PSUM") as ps_pool, \
             tc.tile_pool(name="ob_pool", bufs=1) as ob_pool:

            # ---- prologue SWDGE loads: chunk 0, scale state, chunks 1-4.
            qw3 = qw.rearrange("(a p) c -> p a c", p=P)
            qts = {}
            qw_insts = {}

            def load_chunk(kc):
                qt = dq.tile([P, NB_O, BCC], I16, name="qt", bufs=3)
                li = nc.gpsimd.dma_start(
                    qt, qw3[:, :, kc * BCC:(kc + 1) * BCC])
                qts[kc] = qt
                qw_insts[kc] = li
                return li

            load_chunk(0)
            am3 = sc_pool.tile([P, NB_O, 64], F32, name="am3")
            nc.gpsimd.dma_start(am3, qam.rearrange("(a p) c -> p a c", p=P))
            cd3 = sc_pool.tile([P, NB_O, 64], F32, name="cd3")
            nc.gpsimd.dma_start(cd3, qcode.rearrange("(a p) c -> p a c", p=P))
            c23 = sc_pool.tile([P, NB_O, 16], F32, name="c23")
            nc.gpsimd.dma_start(c23, c2.rearrange("(a p) c -> p a c", p=P))
            am23 = sc_pool.tile([P, NB_O, 16], F32, name="am23")
            nc.gpsimd.dma_start(am23, am2.rearrange("(a p) c -> p a c", p=P))
            of3 = sc_pool.tile([P, NB_O, 64], F32, name="of3")
            nc.gpsimd.dma_start(of3, qoff.rearrange("(a p) c -> p a c", p=P))
            for kc in range(1, 5):
                load_chunk(kc)

            # ---- warmup matmuls on zeroed tiles (PE busy from ~7us, HAM
            # warm when real matmuls start at ~38us).
            wz = sc_pool.tile([P, P], F16, name="wz")
            nc.vector.memset(wz, 0.0)
            ww = sc_pool.tile([P, O_C], F16, name="ww")
            nc.vector.memset(ww, 0.0)
            wps = ps_pool.tile([P, O_C], F32, name="ps")
            for _ in range(N_WARM0):
                nc.tensor.matmul(wps, wz, ww, start=True, stop=True)

            # ---- scale prep (DVE):  S = (am/code) * (am2/c2) as fp16,
            # offS = off*S
            rc = sc_pool.tile([P, NB_O, 64], F32, name="rc")
            nc.vector.reciprocal_approx_fast(rc, cd3)
            s1 = sc_pool.tile([P, NB_O, 64], F32, name="s1")
            nc.vector.tensor_tensor(s1, am3, rc, Alu.mult)
            rc2 = sc_pool.tile([P, NB_O, 16], F32, name="rc2")
            nc.vector.reciprocal_approx_fast(rc2, c23)
            s2 = sc_pool.tile([P, NB_O, 16], F32, name="s2")
            nc.vector.tensor_tensor(s2, am23, rc2, Alu.mult)
            S3 = sc_pool.tile([P, NB_O, 64], F16, name="S3")
            nc.vector.tensor_tensor(
                S3, s1, s2.unsqueeze(3).broadcast_to([P, NB_O, 16, 4]), Alu.mult)
            offS3 = sc_pool.tile([P, NB_O, 64], F16, name="offS3")
            nc.vector.tensor_tensor(offS3, of3, S3, Alu.mult)

            # ---- dequant + W round-trip, chunk-major, batched ops ----
            wts, wt_insts, wd_insts = [], [], []
            for kc in range(NKC):
                if kc + 4 < NKC:
                    load_chunk(kc + 4)
                wd = wdram.tile([O_C, KCW], F16, name=f"wd{kc}")
                w_nat = dq.tile([P, NB_O, KCW], F16, name="wn", bufs=3)
                qt = qts[kc]
                hi = dq.tile([P, NB_O, BCC], I16, name="hi", bufs=1)
                nc.vector.tensor_scalar(hi, qt, 4, None,
                                        Alu.logical_shift_right)
                lo = dq.tile([P, NB_O, BCC], F16, name="lo")
                nc.vector.scalar_tensor_tensor(
                    lo, hi, -16.0, qt, Alu.mult, Alu.add)
                sb = S3[:, :, kc * NBC:(kc + 1) * NBC] \
                    .unsqueeze(3).broadcast_to([P, NB_O, NBC, 32])
                mlo = dq.tile([P, NB_O, BCC], F16, name="mlo")
                nc.vector.tensor_tensor(mlo, lo, sb, Alu.mult)
                mhi = dq.tile([P, NB_O, BCC], F16, name="mhi")
                nc.vector.tensor_tensor(mhi, hi, sb, Alu.mult)
                offs = offS3[:, :, kc * NBC:(kc + 1) * NBC] \
                    .unsqueeze(3).broadcast_to([P, NB_O, NBC, 32])
                nc.vector.tensor_tensor(w_nat[:, :, 0::2], mlo, offs,
                                        Alu.subtract)
                nc.vector.tensor_tensor(w_nat[:, :, 1::2], mhi, offs,
                                        Alu.subtract)
                qts.pop(kc)
                di = nc.gpsimd.dma_start(
                    wd[:, :].rearrange("(a p) c -> p a c", p=P), w_nat)
                wt = wt_pool.tile([P, KKC, O_C], F16, name=f"wt{kc}")
                wi = nc.scalar.dma_start(out=wt, in_=wd[:, :], transpose=True)
                wts.append(wt)
                wt_insts.append(wi)
                wd_insts.append(di)

            # ---- ramp x transposes (4MB blocks of 512 tokens) ----
            xtr, xtr_insts = [], []
            for rb in range(NRB):
                t = xt_pool.tile([P, KT, TB], F16, name=f"xtr{rb}", bufs=1)
                ti = nc.scalar.dma_start(
                    out=t, in_=x[rb * TB:(rb + 1) * TB, :], transpose=True)
                xtr.append(t)
                xtr_insts.append(ti)

            # ---- ramp matmuls in conveyor-readiness order, with a warmup
            # train covering the x1-transpose window ----
            rps = [[ps_pool.tile([P, O_C], F32, name="ps")
                    for st in range(TB // P)] for rb in range(NRB)]
            groups = sorted(
                ((max(WT_POS[kc], X_POS[rb]), kc, rb)
                 for kc in range(NKC) for rb in range(NRB)))
            warm1_done = False
            for key, kc, rb in groups:
                if key >= X_POS[1] and not warm1_done:
                    warm1_done = True
                    for _ in range(N_WARM1):
                        nc.tensor.matmul(wps, wz, ww, start=True, stop=True)
                for st in range(TB // P):
                    for j in range(KKC):
                        kk = kc * KKC + j
                        nc.tensor.matmul(
                            rps[rb][st],
                            xtr[rb][:, kk, st * P:(st + 1) * P],
                            wts[kc][:, j, :],
                            start=(kk == 0),
                            stop=(kk == KT - 1),
                        )
            for rb in range(NRB):
                ob = ob_pool.tile([P, TB // P, O_C], F16, name="ob")
                for st in range(TB // P):
                    nc.vector.tensor_copy(ob[:, st, :], rps[rb][st])
                r0 = rb * TB
                nc.gpsimd.dma_start(
                    out[r0:r0 + TB, :].rearrange("(st p) c -> p st c", p=P),
                    ob)

            # ---- steady blocks ----
            base = NRB * TB
            xt_insts = []
            for tb in range(n_steady):
                r0 = base + tb * TB
                xt = xt_pool.tile([P, KT, TB], F16, name="xt")
                xi = nc.scalar.dma_start(
                    out=xt, in_=x[r0:r0 + TB, :], transpose=True)
                xt_insts.append(xi)
                last = (tb == n_steady - 1)
                ob = ob_pool.tile([P, TB // P, O_C], F16, name="ob")
                for st in range(TB // P):
                    ps = ps_pool.tile([P, O_C], F32, name="ps")
                    for kk in range(KT):
                        nc.tensor.matmul(
                            ps,
                            xt[:, kk, st * P:(st + 1) * P],
                            wts[kk // KKC][:, kk % KKC, :],
                            start=(kk == 0),
                            stop=(kk == KT - 1),
                        )
                    nc.vector.tensor_copy(ob[:, st, :], ps)
                    if last:
                        # store per 128-token tile to shorten the drain
                        nc.gpsimd.dma_start(
                            out[r0 + st * P:r0 + (st + 1) * P, :],
                            ob[:, st, :])
                if not last:
                    nc.gpsimd.dma_start(
                        out[r0:r0 + TB, :].rearrange("(st p) c -> p st c", p=P),
                        ob)

            # ---- pin the full conveyor ----
            chain = [
                xtr_insts[0],                       # x0
                wd_insts[0], wt_insts[0],           # wd0 wt0
                wd_insts[1], wt_insts[1],           # wd1 wt1
                qw_insts[5],
                xtr_insts[1],                       # x1
                qw_insts[6], qw_insts[7],
            ]
            for kc in range(2, NKC):
                chain += [wd_insts[kc], wt_insts[kc]]
            chain += xt_insts
            for a, b in zip(chain[1:], chain):
                tile_rust_add_dep(a.ins, b.ins, True, "conveyor order")
            tile_rust_add_dep(chain[0].ins, qw_insts[4].ins, True,
                              "prologue first")

    nc.compile()
    return nc


_NC_CACHE = {}


def _get_nc(tokens=TOKENS):
    if tokens not in _NC_CACHE:
        _NC_CACHE[tokens] = _build(tokens)
    return _NC_CACHE[tokens]


def _shard(inputs):
    x = np.ascontiguousarray(np.asarray(inputs["x"], dtype=np.float16))
    qw = np.asarray(inputs["quantized_weight"], dtype=np.int32)
    qam = np.asarray(inputs["quant_absmax"], dtype=np.int32)
    qcode = np.asarray(inputs["quant_code"], dtype=np.float32)
    qoff = np.asarray(inputs["quant_offset"], dtype=np.float32)
    am2 = np.asarray(inputs["state2_absmax"], dtype=np.float32)
    c2 = np.asarray(inputs["state2_code"], dtype=np.float32)

    pb = O_C * BPR        # packed bytes per core
    nb1 = O_C * 64        # primary blocks per core
    nb2 = O_C * 16        # secondary blocks per core
    in_maps = []
    for c in range(N_CORES):
        in_maps.append({
            "x": x,
            "qw": np.ascontiguousarray(
                qw[c * pb:(c + 1) * pb].reshape(O_C, BPR)),
            "qam": np.ascontiguousarray(
                qam[c * nb1:(c + 1) * nb1].reshape(O_C, 64)),
            "qcode": np.ascontiguousarray(
                qcode[c * nb1:(c + 1) * nb1].reshape(O_C, 64)),
            "qoff": np.ascontiguousarray(
                qoff[c * nb1:(c + 1) * nb1].reshape(O_C, 64)),
            "am2": np.ascontiguousarray(
                am2[c * nb2:(c + 1) * nb2].reshape(O_C, 16)),
            "c2": np.ascontiguousarray(
                c2[c * nb2:(c + 1) * nb2].reshape(O_C, 16)),
        })
    return in_maps


def _run(inputs, trace=False, trace_cores=None):
    nc = _get_nc()
    in_maps = _shard(inputs)
    res = run_bass_kernel_spmd(
        nc, in_maps, list(range(N_CORES)), trace=trace,
        trace_cores=trace_cores)
    out = np.concatenate([r["out"] for r in res.results], axis=1)
    return out, res


def kernel(**inputs) -> np.ndarray:
    out, _ = _run(inputs, trace=False)
    return out


# revision 10
# speedup vs baseline: 1.0182x; 1.0182x over previous
"""NF4-style 4-bit quantized linear: out = x @ dequant(w).T on 8 TRN2 NeuronCores.

Column-parallel sharding: core c owns output features [c*512, (c+1)*512) and the
corresponding contiguous slices of the packed weight + quant state arrays; x is
replicated. The Tile scheduler serializes ALL DMA traffic against in-flight
xbar transposes and each serialized link pays ~2us of completion latency, so
the kernel minimizes conveyor links:
  prologue: packed-weight loads (gpsimd, int32->int16 cast) run in parallel
  with scale-state loads (sync) before any transpose; then the pinned conveyor
    x0 -> wd0 wt0 -> x1 -> wd1 wt1 -> wd23 wt23 -> wd47 wt47 -> xt0 xt1 ...
  where W round-trips DRAM in 4 pieces (chunks 0, 1, 2-3, 4-7) sized so late
  pieces amortize the per-link latency while early pieces start the PE fast.
Dequant (8 chunks of 4 k-tiles, 6 batched DVE ops each) runs in the shadow of
the x transposes. The first two 512-token blocks form the ramp (8 psums,
matmuls emitted in conveyor-readiness order); warmup-matmul trains on zeroed
tiles cover the pre-x0 window and the x1-transpose gap so the PE's HAM clock
gate never drops to half rate. The x tiles share one bufs=3 pool; output
stores are batched per block, the last block stores per 128-token tile.
"""
import numpy as np

import concourse.bass as bass
import concourse.mybir as mybir
import concourse.tile as tile
from concourse import bacc
from concourse.tile_rust import add_dep_helper as tile_rust_add_dep
from concourse.bass_utils import run_bass_kernel_spmd

F16 = mybir.dt.float16
F32 = mybir.dt.float32
I32 = mybir.dt.int32
I16 = mybir.dt.int16
Alu = mybir.AluOpType

P = 128
TOKENS = 8192
IN_F = 4096
OUT_F = 4096
N_CORES = 8
O_C = OUT_F // N_CORES          # 512 out features per core
KT = IN_F // P                  # 32 k-tiles
BPR = IN_F // 2                 # 2048 packed bytes per weight row
NB_O = O_C // P                 # 4 o-tiles of 128 rows
TB = 512                        # token block (4MB transpose)

NKC = 8                         # W chunks
KKC = KT // NKC                 # 4 k-tiles per chunk
KCW = IN_F // NKC               # 512 k values per chunk
BCC = BPR // NKC                # 256 packed bytes per chunk (per row)
NBC = BCC // 32                 # 8 quant blocks per chunk (per row)

NRB = 2                         # ramp blocks (2*512 tokens = 8 psums)

N_WARM0 = 135                   # warmup matmuls before the first real matmul
N_WARM1 = 44                    # warmup matmuls covering the x1 window

# W piece of each chunk: (piece index, k-tile offset within piece)
PIECE = [0, 1, 2, 2, 3, 3, 3, 3]
PKOFF = [0, 0, 0, KKC, 0, KKC, 2 * KKC, 3 * KKC]
PIECE_CHUNKS = [[0], [1], [2, 3], [4, 5, 6, 7]]

# conveyor positions for readiness-ordering the ramp matmuls:
# x0=0, wd0=1, wt0=2, x1=3, wd1=4, wt1=5, wd23=6, wt23=7, wd47=8, wt47=9
X_POS = [0, 3]
WTP_POS = [2, 5, 7, 9]          # per W piece


def _build(tokens=TOKENS):
    nc = bacc.Bacc("TRN2", target_bir_lowering=False, debug=False,
                   enable_asserts=False)

    x = nc.dram_tensor("x", [tokens, IN_F], F16, kind="ExternalInput").ap()
    qw = nc.dram_tensor("qw", [O_C, BPR], I32, kind="ExternalInput").ap()
    qam = nc.dram_tensor("qam", [O_C, 64], I32, kind="ExternalInput").ap()
    qcode = nc.dram_tensor("qcode", [O_C, 64], F32, kind="ExternalInput").ap()
    qoff = nc.dram_tensor("qoff", [O_C, 64], F32, kind="ExternalInput").ap()
    am2 = nc.dram_tensor("am2", [O_C, 16], F32, kind="ExternalInput").ap()
    c2 = nc.dram_tensor("c2", [O_C, 16], F32, kind="ExternalInput").ap()
    out = nc.dram_tensor("out", [tokens, O_C], F16, kind="ExternalOutput").ap()

    n_steady = tokens // TB - NRB

    with tile.TileContext(nc) as tc:
        with tc.tile_pool(name="wt_pool", bufs=1) as wt_pool, \
             tc.tile_pool(name="wdram", bufs=1, space="DRAM") as wdram, \
             tc.tile_pool(name="sc_pool", bufs=1) as sc_pool, \
             tc.tile_pool(name="dq", bufs=1) as dq, \
             tc.tile_pool(name="xt_pool", bufs=3) as xt_pool, \
             tc.tile_pool(name="ps_pool", bufs=8, space="PSUM") as ps_pool, \
             tc.tile_pool(name="ob_pool", bufs=1) as ob_pool:

            # ---- prologue: all packed-weight chunk loads on the SWDGE
            # (gpsimd) queue, all scale-state loads on the sync HWDGE queue,
            # running in parallel before any transpose.
            qw3 = qw.rearrange("(a p) c -> p a c", p=P)
            qt_tiles, qw_insts = [], []
            for kc in range(NKC):
                qt = dq.tile([P, NB_O, BCC], I16, name=f"qt{kc}")
                li = nc.gpsimd.dma_start(
                    qt, qw3[:, :, kc * BCC:(kc + 1) * BCC])
                qt_tiles.append(qt)
                qw_insts.append(li)
            am3 = sc_pool.tile([P, NB_O, 64], I32, name="am3")
            nc.sync.dma_start(am3, qam.rearrange("(a p) c -> p a c", p=P))
            cd3 = sc_pool.tile([P, NB_O, 64], F32, name="cd3")
            nc.sync.dma_start(cd3, qcode.rearrange("(a p) c -> p a c", p=P))
            c23 = sc_pool.tile([P, NB_O, 16], F32, name="c23")
            nc.sync.dma_start(c23, c2.rearrange("(a p) c -> p a c", p=P))
            am23 = sc_pool.tile([P, NB_O, 16], F32, name="am23")
            nc.sync.dma_start(am23, am2.rearrange("(a p) c -> p a c", p=P))
            of3 = sc_pool.tile([P, NB_O, 64], F32, name="of3")
            last_sc = nc.sync.dma_start(
                of3, qoff.rearrange("(a p) c -> p a c", p=P))

            # ---- warmup matmuls on zeroed tiles (PE busy from ~7us, HAM
            # warm when real matmuls start at ~40us).
            wz = sc_pool.tile([P, P], F16, name="wz")
            nc.vector.memset(wz, 0.0)
            ww = sc_pool.tile([P, O_C], F16, name="ww")
            nc.vector.memset(ww, 0.0)
            wps = ps_pool.tile([P, O_C], F32, name="ps")
            for _ in range(N_WARM0):
                nc.tensor.matmul(wps, wz, ww, start=True, stop=True)

            # ---- scale prep (DVE):  S = (am/code) * (am2/c2) as fp16,
            # offS = off*S
            rc = sc_pool.tile([P, NB_O, 64], F32, name="rc")
            nc.vector.reciprocal_approx_fast(rc, cd3)
            s1 = sc_pool.tile([P, NB_O, 64], F32, name="s1")
            nc.vector.tensor_tensor(s1, am3, rc, Alu.mult)
            rc2 = sc_pool.tile([P, NB_O, 16], F32, name="rc2")
            nc.vector.reciprocal_approx_fast(rc2, c23)
            s2 = sc_pool.tile([P, NB_O, 16], F32, name="s2")
            nc.vector.tensor_tensor(s2, am23, rc2, Alu.mult)
            S3 = sc_pool.tile([P, NB_O, 64], F16, name="S3")
            nc.vector.tensor_tensor(
                S3, s1, s2.unsqueeze(3).broadcast_to([P, NB_O, 16, 4]), Alu.mult)
            offS3 = sc_pool.tile([P, NB_O, 64], F16, name="offS3")
            nc.vector.tensor_tensor(offS3, of3, S3, Alu.mult)

            # ---- dequant into per-piece natural tiles ----
            wn = [dq.tile([P, NB_O, KCW * len(PIECE_CHUNKS[pi])], F16,
                          name=f"wn{pi}") for pi in range(4)]
            for kc in range(NKC):
                qt = qt_tiles[kc]
                off = PKOFF[kc] * P  # k offset within the piece tile
                w_nat = wn[PIECE[kc]]
                hi = dq.tile([P, NB_O, BCC], I16, name="hi")
                nc.vector.tensor_scalar(hi, qt, 4, None,
                                        Alu.logical_shift_right)
                lo = dq.tile([P, NB_O, BCC], F16, name="lo")
                nc.vector.scalar_tensor_tensor(
                    lo, hi, -16.0, qt, Alu.mult, Alu.add)
                sb = S3[:, :, kc * NBC:(kc + 1) * NBC] \
                    .unsqueeze(3).broadcast_to([P, NB_O, NBC, 32])
                mlo = dq.tile([P, NB_O, BCC], F16, name="mlo")
                nc.vector.tensor_tensor(mlo, lo, sb, Alu.mult)
                mhi = dq.tile([P, NB_O, BCC], F16, name="mhi")
                nc.vector.tensor_tensor(mhi, hi, sb, Alu.mult)
                offs = offS3[:, :, kc * NBC:(kc + 1) * NBC] \
                    .unsqueeze(3).broadcast_to([P, NB_O, NBC, 32])
                nc.vector.tensor_tensor(
                    w_nat[:, :, off:off + KCW:2], mlo, offs, Alu.subtract)
                nc.vector.tensor_tensor(
                    w_nat[:, :, off + 1:off + KCW:2], mhi, offs, Alu.subtract)

            # ---- W piece round-trips (store + transpose-load) ----
            wtp, wd_insts, wt_insts = [], [], []
            for pi in range(4):
                nkt = KKC * len(PIECE_CHUNKS[pi])
                wd = wdram.tile([O_C, nkt * P], F16, name=f"wd{pi}")
                di = nc.gpsimd.dma_start(
                    wd[:, :].rearrange("(a p) c -> p a c", p=P), wn[pi])
                wt = wt_pool.tile([P, nkt, O_C], F16, name=f"wt{pi}")
                wi = nc.scalar.dma_start(out=wt, in_=wd[:, :], transpose=True)
                wtp.append(wt)
                wd_insts.append(di)
                wt_insts.append(wi)

            def wt_ap(kk):
                kc, j = kk // KKC, kk % KKC
                return wtp[PIECE[kc]][:, PKOFF[kc] + j, :]

            # ---- ramp x transposes (shared bufs=3 pool with steady) ----
            xtr, xtr_insts = [], []
            for rb in range(NRB):
                t = xt_pool.tile([P, KT, TB], F16, name="xt")
                ti = nc.scalar.dma_start(
                    out=t, in_=x[rb * TB:(rb + 1) * TB, :], transpose=True)
                xtr.append(t)
                xtr_insts.append(ti)

            # ---- ramp matmuls in conveyor-readiness order, with a warmup
            # train covering the x1-transpose window ----
            rps = [[ps_pool.tile([P, O_C], F32, name="ps")
                    for st in range(TB // P)] for rb in range(NRB)]
            groups = sorted(
                ((max(WTP_POS[PIECE[kc]], X_POS[rb]), kc, rb)
                 for kc in range(NKC) for rb in range(NRB)))
            warm1_done = False
            for key, kc, rb in groups:
                if key >= X_POS[1] and not warm1_done:
                    warm1_done = True
                    for _ in range(N_WARM1):
                        nc.tensor.matmul(wps, wz, ww, start=True, stop=True)
                for st in range(TB // P):
                    for j in range(KKC):
                        kk = kc * KKC + j
                        nc.tensor.matmul(
                            rps[rb][st],
                            xtr[rb][:, kk, st * P:(st + 1) * P],
                            wt_ap(kk),
                            start=(kk == 0),
                            stop=(kk == KT - 1),
                        )
            for rb in range(NRB):
                ob = ob_pool.tile([P, TB // P, O_C], F16, name="ob")
                for st in range(TB // P):
                    nc.vector.tensor_copy(ob[:, st, :], rps[rb][st])
                r0 = rb * TB
                nc.gpsimd.dma_start(
                    out[r0:r0 + TB, :].rearrange("(st p) c -> p st c", p=P),
                    ob)

            # ---- steady blocks ----
            base = NRB * TB
            xt_insts = []
            for tb in range(n_steady):
                r0 = base + tb * TB
                xt = xt_pool.tile([P, KT, TB], F16, name="xt")
                xi = nc.scalar.dma_start(
                    out=xt, in_=x[r0:r0 + TB, :], transpose=True)
                xt_insts.append(xi)
                last = (tb == n_steady - 1)
                ob = ob_pool.tile([P, TB // P, O_C], F16, name="ob")
                for st in range(TB // P):
                    ps = ps_pool.tile([P, O_C], F32, name="ps")
                    for kk in range(KT):
                        nc.tensor.matmul(
                            ps,
                            xt[:, kk, st * P:(st + 1) * P],
                            wt_ap(kk),
                            start=(kk == 0),
                            stop=(kk == KT - 1),
                        )
                    nc.vector.tensor_copy(ob[:, st, :], ps)
                    if last:
                        # store per 128-token tile to shorten the drain
                        nc.gpsimd.dma_start(
                            out[r0 + st * P:r0 + (st + 1) * P, :],
                            ob[:, st, :])
                if not last:
                    nc.gpsimd.dma_start(
                        out[r0:r0 + TB, :].rearrange("(st p) c -> p st c", p=P),
                        ob)

            # ---- pin the conveyor ----
            chain = [
                xtr_insts[0],
                wd_insts[0], wt_insts[0],
                xtr_insts[1],
                wd_insts[1], wt_insts[1],
                wd_insts[2], wt_insts[2],
                wd_insts[3], wt_insts[3],
            ]
            chain += xt_insts
            for a, b in zip(chain[1:], chain):
                tile_rust_add_dep(a.ins, b.ins, True, "conveyor order")
            tile_rust_add_dep(chain[0].ins, qw_insts[-1].ins, True,
                              "prologue first")
            tile_rust_add_dep(chain[0].ins, last_sc.ins, True,
                              "prologue first")

    nc.compile()
    return nc


_NC_CACHE = {}


def _get_nc(tokens=TOKENS):
    if tokens not in _NC_CACHE:
        _NC_CACHE[tokens] = _build(tokens)
    return _NC_CACHE[tokens]


def _shard(inputs):
    x = np.ascontiguousarray(np.asarray(inputs["x"], dtype=np.float16))
    qw = np.asarray(inputs["quantized_weight"], dtype=np.int32)
    qam = np.asarray(inputs["quant_absmax"], dtype=np.int32)
    qcode = np.asarray(inputs["quant_code"], dtype=np.float32)
    qoff = np.asarray(inputs["quant_offset"], dtype=np.float32)
    am2 = np.asarray(inputs["state2_absmax"], dtype=np.float32)
    c2 = np.asarray(inputs["state2_code"], dtype=np.float32)

    pb = O_C * BPR        # packed bytes per core
    nb1 = O_C * 64        # primary blocks per core
    nb2 = O_C * 16        # secondary blocks per core
    in_maps = []
    for c in range(N_CORES):
        in_maps.append({
            "x": x,
            "qw": np.ascontiguousarray(
                qw[c * pb:(c + 1) * pb].reshape(O_C, BPR)),
            "qam": np.ascontiguousarray(
                qam[c * nb1:(c + 1) * nb1].reshape(O_C, 64)),
            "qcode": np.ascontiguousarray(
                qcode[c * nb1:(c + 1) * nb1].reshape(O_C, 64)),
            "qoff": np.ascontiguousarray(
                qoff[c * nb1:(c + 1) * nb1].reshape(O_C, 64)),
            "am2": np.ascontiguousarray(
                am2[c * nb2:(c + 1) * nb2].reshape(O_C, 16)),
            "c2": np.ascontiguousarray(
                c2[c * nb2:(c + 1) * nb2].reshape(O_C, 16)),
        })
    return in_maps


def _run(inputs, trace=False, trace_cores=None):
    nc = _get_nc()
    in_maps = _shard(inputs)
    res = run_bass_kernel_spmd(
        nc, in_maps, list(range(N_CORES)), trace=trace,
        trace_cores=trace_cores)
    out = np.concatenate([r["out"] for r in res.results], axis=1)
    return out, res


def kernel(**inputs) -> np.ndarray:
    out, _ = _run(inputs, trace=False)
    return out


# revision 15
# speedup vs baseline: 1.0447x; 1.0260x over previous
"""NF4-style 4-bit quantized linear: out = x @ dequant(w).T on 8 TRN2 NeuronCores.

Column-parallel sharding: core c owns output features [c*512, (c+1)*512) and the
corresponding contiguous slices of the packed weight + quant state arrays; x is
replicated. The Tile scheduler serializes ALL DMA traffic against in-flight
xbar transposes and each serialized link pays ~2us of completion latency, so
the kernel minimizes conveyor links:
  prologue: packed-weight loads (gpsimd, int32->int16 cast) run in parallel
  with scale-state loads (sync) before any transpose; then the pinned conveyor
    x0 -> wd0 wt0 -> x1 -> wd1 wt1 -> wd23 wt23 -> wd47 wt47 -> xt0 xt1 ...
  where W round-trips DRAM in 4 pieces (chunks 0, 1, 2-3, 4-7) sized so late
  pieces amortize the per-link latency while early pieces start the PE fast.
Dequant (8 chunks of 4 k-tiles, 6 batched DVE ops each) runs in the shadow of
the x transposes. The first two 512-token blocks form the ramp (8 psums,
matmuls emitted in conveyor-readiness order); warmup-matmul trains on zeroed
tiles cover the pre-x0 window and the x1-transpose gap so the PE's HAM clock
gate never drops to half rate. The x tiles share one bufs=3 pool; output
stores are batched per block, the last block stores per 128-token tile.
"""
import numpy as np

import concourse.bass as bass
import concourse.mybir as mybir
import concourse.tile as tile
from concourse import bacc
from concourse.tile_rust import add_dep_helper as tile_rust_add_dep
from concourse.bass_utils import run_bass_kernel_spmd

F16 = mybir.dt.float16
F32 = mybir.dt.float32
I32 = mybir.dt.int32
I16 = mybir.dt.int16
Alu = mybir.AluOpType

P = 128
TOKENS = 8192
IN_F = 4096
OUT_F = 4096
N_CORES = 8
O_C = OUT_F // N_CORES          # 512 out features per core
KT = IN_F // P                  # 32 k-tiles
BPR = IN_F // 2                 # 2048 packed bytes per weight row
NB_O = O_C // P                 # 4 o-tiles of 128 rows
TB = 512                        # token block (4MB transpose)

NKC = 8                         # W chunks
KKC = KT // NKC                 # 4 k-tiles per chunk
KCW = IN_F // NKC               # 512 k values per chunk
BCC = BPR // NKC                # 256 packed bytes per chunk (per row)
NBC = BCC // 32                 # 8 quant blocks per chunk (per row)

NRB = 2                         # ramp blocks (2*512 tokens = 8 psums)

N_WARM0 = 135                   # warmup matmuls before the first real matmul
N_WARM1 = 44                    # warmup matmuls covering the x1 window

# W piece of each chunk: (piece index, k-tile offset within piece)
PIECE = [0, 1, 2, 2, 3, 3, 3, 3]
PKOFF = [0, 0, 0, KKC, 0, KKC, 2 * KKC, 3 * KKC]
PIECE_CHUNKS = [[0], [1], [2, 3], [4, 5, 6, 7]]

# conveyor positions for readiness-ordering the ramp matmuls:
# x0=0, wd0=1, wt0=2, x1=3, wd1=4, wt1=5, wd23=6, wt23=7, wd47=8, wt47=9
X_POS = [0, 3]
WTP_POS = [2, 5, 7, 9]          # per W piece


def _build(tokens=TOKENS):
    nc = bacc.Bacc("TRN2", target_bir_lowering=False, debug=False,
                   enable_asserts=False)

    x = nc.dram_tensor("x", [tokens, IN_F], F16, kind="ExternalInput").ap()
    qw = nc.dram_tensor("qw", [O_C, BPR], I32, kind="ExternalInput").ap()
    qam = nc.dram_tensor("qam", [O_C, 64], I32, kind="ExternalInput").ap()
    qcode = nc.dram_tensor("qcode", [O_C, 64], F32, kind="ExternalInput").ap()
    qoff = nc.dram_tensor("qoff", [O_C, 64], F32, kind="ExternalInput").ap()
    am2 = nc.dram_tensor("am2", [O_C, 16], F32, kind="ExternalInput").ap()
    c2 = nc.dram_tensor("c2", [O_C, 16], F32, kind="ExternalInput").ap()
    out = nc.dram_tensor("out", [tokens, O_C], F16, kind="ExternalOutput").ap()

    n_steady = tokens // TB - NRB

    with tile.TileContext(nc) as tc:
        with tc.tile_pool(name="wt_pool", bufs=1) as wt_pool, \
             tc.tile_pool(name="wdram", bufs=1, space="DRAM") as wdram, \
             tc.tile_pool(name="sc_pool", bufs=1) as sc_pool, \
             tc.tile_pool(name="dq", bufs=1) as dq, \
             tc.tile_pool(name="xt_pool", bufs=3) as xt_pool, \
             tc.tile_pool(name="ps_pool", bufs=8, space="PSUM") as ps_pool, \
             tc.tile_pool(name="ob_pool", bufs=1) as ob_pool:

            # ---- prologue: all packed-weight chunk loads on the SWDGE
            # (gpsimd) queue, all scale-state loads on the sync HWDGE queue,
            # running in parallel before any transpose.
            qw3 = qw.rearrange("(a p) c -> p a c", p=P)
            qtall = dq.tile([P, NB_O, BPR], I16, name="qtall")
            qw_li = nc.gpsimd.dma_start(qtall, qw3)

            def qt_slice(kc):
                return qtall[:, :, kc * BCC:(kc + 1) * BCC]
            am3 = sc_pool.tile([P, NB_O, 64], I32, name="am3")
            nc.sync.dma_start(am3, qam.rearrange("(a p) c -> p a c", p=P))
            cd3 = sc_pool.tile([P, NB_O, 64], F32, name="cd3")
            nc.sync.dma_start(cd3, qcode.rearrange("(a p) c -> p a c", p=P))
            c23 = sc_pool.tile([P, NB_O, 16], F32, name="c23")
            nc.sync.dma_start(c23, c2.rearrange("(a p) c -> p a c", p=P))
            am23 = sc_pool.tile([P, NB_O, 16], F32, name="am23")
            nc.sync.dma_start(am23, am2.rearrange("(a p) c -> p a c", p=P))
            of3 = sc_pool.tile([P, NB_O, 64], F32, name="of3")
            last_sc = nc.sync.dma_start(
                of3, qoff.rearrange("(a p) c -> p a c", p=P))

            # ---- warmup matmuls on zeroed tiles (PE busy from ~7us, HAM
            # warm when real matmuls start at ~40us).
            wz = sc_pool.tile([P, P], F16, name="wz")
            nc.vector.memset(wz, 0.0)
            ww = sc_pool.tile([P, O_C], F16, name="ww")
            nc.vector.memset(ww, 0.0)
            wps = ps_pool.tile([P, O_C], F32, name="ps")
            for _ in range(N_WARM0):
                nc.tensor.matmul(wps, wz, ww, start=True, stop=True)

            # ---- scale prep (DVE):  S = (am/code) * (am2/c2) as fp16,
            # offS = off*S
            rc = sc_pool.tile([P, NB_O, 64], F32, name="rc")
            nc.vector.reciprocal_approx_fast(rc, cd3)
            s1 = sc_pool.tile([P, NB_O, 64], F32, name="s1")
            nc.vector.tensor_tensor(s1, am3, rc, Alu.mult)
            rc2 = sc_pool.tile([P, NB_O, 16], F32, name="rc2")
            nc.vector.reciprocal_approx_fast(rc2, c23)
            s2 = sc_pool.tile([P, NB_O, 16], F32, name="s2")
            nc.vector.tensor_tensor(s2, am23, rc2, Alu.mult)
            S3 = sc_pool.tile([P, NB_O, 64], F16, name="S3")
            nc.vector.tensor_tensor(
                S3, s1, s2.unsqueeze(3).broadcast_to([P, NB_O, 16, 4]), Alu.mult)
            offS3 = sc_pool.tile([P, NB_O, 64], F16, name="offS3")
            nc.vector.tensor_tensor(offS3, of3, S3, Alu.mult)

            # ---- dequant into per-piece natural tiles ----
            wn = [dq.tile([P, NB_O, KCW * len(PIECE_CHUNKS[pi])], F16,
                          name=f"wn{pi}") for pi in range(4)]
            for kc in range(NKC):
                qt = qt_slice(kc)
                off = PKOFF[kc] * P  # k offset within the piece tile
                w_nat = wn[PIECE[kc]]
                hi = dq.tile([P, NB_O, BCC], I16, name="hi")
                nc.vector.tensor_scalar(hi, qt, 4, None,
                                        Alu.logical_shift_right)
                lo = dq.tile([P, NB_O, BCC], F16, name="lo")
                nc.vector.scalar_tensor_tensor(
                    lo, hi, -16.0, qt, Alu.mult, Alu.add)
                sb = S3[:, :, kc * NBC:(kc + 1) * NBC] \
                    .unsqueeze(3).broadcast_to([P, NB_O, NBC, 32])
                mlo = dq.tile([P, NB_O, BCC], F16, name="mlo")
                nc.vector.tensor_tensor(mlo, lo, sb, Alu.mult)
                mhi = dq.tile([P, NB_O, BCC], F16, name="mhi")
                nc.vector.tensor_tensor(mhi, hi, sb, Alu.mult)
                offs = offS3[:, :, kc * NBC:(kc + 1) * NBC] \
                    .unsqueeze(3).broadcast_to([P, NB_O, NBC, 32])
                nc.vector.tensor_tensor(
                    w_nat[:, :, off:off + KCW:2], mlo, offs, Alu.subtract)
                nc.vector.tensor_tensor(
                    w_nat[:, :, off + 1:off + KCW:2], mhi, offs, Alu.subtract)

            # ---- W piece round-trips (store + transpose-load) ----
            wtp, wd_insts, wt_insts = [], [], []
            for pi in range(4):
                nkt = KKC * len(PIECE_CHUNKS[pi])
                wd = wdram.tile([O_C, nkt * P], F16, name=f"wd{pi}")
                di = nc.gpsimd.dma_start(
                    wd[:, :].rearrange("(a p) c -> p a c", p=P), wn[pi])
                wt = wt_pool.tile([P, nkt, O_C], F16, name=f"wt{pi}")
                wi = nc.scalar.dma_start(out=wt, in_=wd[:, :], transpose=True)
                wtp.append(wt)
                wd_insts.append(di)
                wt_insts.append(wi)

            def wt_ap(kk):
                kc, j = kk // KKC, kk % KKC
                return wtp[PIECE[kc]][:, PKOFF[kc] + j, :]

            # ---- ramp x transposes (shared bufs=3 pool with steady) ----
            xtr, xtr_insts = [], []
            for rb in range(NRB):
                t = xt_pool.tile([P, KT, TB], F16, name="xt")
                ti = nc.scalar.dma_start(
                    out=t, in_=x[rb * TB:(rb + 1) * TB, :], transpose=True)
                xtr.append(t)
                xtr_insts.append(ti)

            # ---- ramp matmuls in conveyor-readiness order, with a warmup
            # train covering the x1-transpose window ----
            rps = [[ps_pool.tile([P, O_C], F32, name="ps")
                    for st in range(TB // P)] for rb in range(NRB)]
            groups = sorted(
                ((max(WTP_POS[PIECE[kc]], X_POS[rb]), kc, rb)
                 for kc in range(NKC) for rb in range(NRB)))
            warm1_done = False
            for key, kc, rb in groups:
                if key >= X_POS[1] and not warm1_done:
                    warm1_done = True
                    for _ in range(N_WARM1):
                        nc.tensor.matmul(wps, wz, ww, start=True, stop=True)
                for st in range(TB // P):
                    for j in range(KKC):
                        kk = kc * KKC + j
                        nc.tensor.matmul(
                            rps[rb][st],
                            xtr[rb][:, kk, st * P:(st + 1) * P],
                            wt_ap(kk),
                            start=(kk == 0),
                            stop=(kk == KT - 1),
                        )
            for rb in range(NRB):
                ob = ob_pool.tile([P, TB // P, O_C], F16, name="ob")
                for st in range(TB // P):
                    nc.vector.tensor_copy(ob[:, st, :], rps[rb][st])
                r0 = rb * TB
                nc.gpsimd.dma_start(
                    out[r0:r0 + TB, :].rearrange("(st p) c -> p st c", p=P),
                    ob)

            # ---- steady blocks ----
            base = NRB * TB
            xt_insts = []
            for tb in range(n_steady):
                r0 = base + tb * TB
                xt = xt_pool.tile([P, KT, TB], F16, name="xt")
                xi = nc.scalar.dma_start(
                    out=xt, in_=x[r0:r0 + TB, :], transpose=True)
                xt_insts.append(xi)
                last = (tb == n_steady - 1)
                ob = ob_pool.tile([P, TB // P, O_C], F16, name="ob")
                for st in range(TB // P):
                    ps = ps_pool.tile([P, O_C], F32, name="ps")
                    for kk in range(KT):
                        nc.tensor.matmul(
                            ps,
                            xt[:, kk, st * P:(st + 1) * P],
                            wt_ap(kk),
                            start=(kk == 0),
                            stop=(kk == KT - 1),
                        )
                    nc.vector.tensor_copy(ob[:, st, :], ps)
                    if last:
                        # store per 128-token tile to shorten the drain
                        nc.gpsimd.dma_start(
                            out[r0 + st * P:r0 + (st + 1) * P, :],
                            ob[:, st, :])
                if not last:
                    nc.gpsimd.dma_start(
                        out[r0:r0 + TB, :].rearrange("(st p) c -> p st c", p=P),
                        ob)

            # ---- pin the conveyor ----
            chain = [
                xtr_insts[0],
                wd_insts[0], wt_insts[0],
                xtr_insts[1],
                wd_insts[1], wt_insts[1],
                wd_insts[2], wt_insts[2],
                wd_insts[3], wt_insts[3],
            ]
            chain += xt_insts
            for a, b in zip(chain[1:], chain):
                tile_rust_add_dep(a.ins, b.ins, True, "conveyor order")
            tile_rust_add_dep(chain[0].ins, qw_li.ins, True,
                              "prologue first")
            tile_rust_add_dep(chain[0].ins, last_sc.ins, True,
                              "prologue first")

    nc.compile()
    return nc


_NC_CACHE = {}


def _get_nc(tokens=TOKENS):
    if tokens not in _NC_CACHE:
        _NC_CACHE[tokens] = _build(tokens)
    return _NC_CACHE[tokens]


def _shard(inputs):
    x = np.ascontiguousarray(np.asarray(inputs["x"], dtype=np.float16))
    qw = np.asarray(inputs["quantized_weight"], dtype=np.int32)
    qam = np.asarray(inputs["quant_absmax"], dtype=np.int32)
    qcode = np.asarray(inputs["quant_code"], dtype=np.float32)
    qoff = np.asarray(inputs["quant_offset"], dtype=np.float32)
    am2 = np.asarray(inputs["state2_absmax"], dtype=np.float32)
    c2 = np.asarray(inputs["state2_code"], dtype=np.float32)

    pb = O_C * BPR        # packed bytes per core
    nb1 = O_C * 64        # primary blocks per core
    nb2 = O_C * 16        # secondary blocks per core
    in_maps = []
    for c in range(N_CORES):
        in_maps.append({
            "x": x,
            "qw": np.ascontiguousarray(
                qw[c * pb:(c + 1) * pb].reshape(O_C, BPR)),
            "qam": np.ascontiguousarray(
                qam[c * nb1:(c + 1) * nb1].reshape(O_C, 64)),
            "qcode": np.ascontiguousarray(
                qcode[c * nb1:(c + 1) * nb1].reshape(O_C, 64)),
            "qoff": np.ascontiguousarray(
                qoff[c * nb1:(c + 1) * nb1].reshape(O_C, 64)),
            "am2": np.ascontiguousarray(
                am2[c * nb2:(c + 1) * nb2].reshape(O_C, 16)),
            "c2": np.ascontiguousarray(
                c2[c * nb2:(c + 1) * nb2].reshape(O_C, 16)),
        })
    return in_maps


def _run(inputs, trace=False, trace_cores=None):
    nc = _get_nc()
    in_maps = _shard(inputs)
    res = run_bass_kernel_spmd(
        nc, in_maps, list(range(N_CORES)), trace=trace,
        trace_cores=trace_cores)
    out = np.concatenate([r["out"] for r in res.results], axis=1)
    return out, res


def kernel(**inputs) -> np.ndarray:
    out, _ = _run(inputs, trace=False)
    return out
